# revision 1
# baseline (speedup 1.0000x reference)
"""4-branch bidirectional GRU (nn_RNN_2817498546846) on 8 TRN2 NeuronCores.

Sharding: core i handles cell k=i//2 (air0,bed0,air1,bed1) and batch half
j=i%2 (256 rows). Weights per-core = one cell only; no collectives.
Cells 2,3 consume the time-reversed sequence -> host reverses their data,
so the device program is identical on all cores (pure SPMD).

Layout: hidden state transposed [H,B]=[512,256] as 4 tiles [128,256] so
gate biases are per-partition ACT operands. Input projection for block b+1
is interleaved into block b's steps; the r/z input-side matmuls (pass 1)
are emitted before the hb-dependent work so PE has filler while the
previous step's elementwise chain drains.
"""

import sys
import numpy as np

sys.path.insert(0, "/opt/trn_rl_repo")

import ml_dtypes

B, F, T, H, K = 512, 64, 64, 512, 4
BL = 256          # batch per core
NBLK, SPB = 8, 8  # 8 blocks x 8 steps
BF16 = ml_dtypes.bfloat16

_CACHE = {}
TRACE = False   # test harness sets True to capture NTFF profile
LAST = {}       # stashes the BassKernelResults of the most recent run


def _legalize(nc, mybir):
    """Walrus codegen allows at most ONE embedded sem wait per instruction
    (libwalrus setupSyncWait asserts count==1 for every ISA struct). Engines
    execute their streams in order, so extra waits move onto same-engine
    NoOps inserted immediately before the offending instruction."""
    n_split = 0
    for f in nc.m.functions:
        for b in f.blocks:
            insts = b.instructions
            out = []
            for ins in insts:
                si = getattr(ins, "sync_info", None)
                waits = list(si.on_wait) if si is not None and si.on_wait else []
                if len(waits) > 1:
                    for k, w in enumerate(waits[:-1]):
                        nop = mybir.InstNoOp(
                            name=f"{ins.name}-lw{k}",
                            engine=ins.engine,
                            bass_nofuse=True,
                            sync_info=mybir.SyncInfo(on_wait=[w], on_update=[]),
                        )
                        out.append(nop)
                        n_split += 1
                    ups = list(si.on_update) if si.on_update else []
                    ins.sync_info = mybir.SyncInfo(on_wait=[waits[-1]], on_update=ups)
                out.append(ins)
            insts[:] = out
    return n_split


def _build():
    import concourse.bass as bass
    import concourse.tile as tile
    from concourse import mybir

    dt = mybir.dt
    AF = mybir.ActivationFunctionType

    nc = bass.Bass("TRN2", target_bir_lowering=False, debug=False, num_devices=8)

    xdat_d = nc.declare_dram_parameter("xdat", [F, T * BL], dt.bfloat16, isOutput=False)
    fcin_d = nc.declare_dram_parameter("fcin", [F, H], dt.bfloat16, isOutput=False)
    fcb_d = nc.declare_dram_parameter("fcb", [128, 4], dt.float32, isOutput=False)
    wih_d = nc.declare_dram_parameter("wih", [H, 3 * H], dt.bfloat16, isOutput=False)
    whh_d = nc.declare_dram_parameter("whh", [H, 3 * H], dt.bfloat16, isOutput=False)
    brz_d = nc.declare_dram_parameter("brz", [128, 8], dt.float32, isOutput=False)
    bni_d = nc.declare_dram_parameter("bni", [128, 4], dt.float32, isOutput=False)
    bnh_d = nc.declare_dram_parameter("bnh", [128, 4], dt.float32, isOutput=False)
    h0_d = nc.declare_dram_parameter("h0", [H, BL], dt.float32, isOutput=False)
    wout_d = nc.declare_dram_parameter("wout", [128, 32 * SPB], dt.bfloat16,
                                       isOutput=False)
    yout_d = nc.declare_dram_parameter("yout", [T, BL], dt.float32, isOutput=True)

    with tile.TileContext(nc) as tc:
        with (
            tc.tile_pool(name="wpool", bufs=1) as wpool,
            tc.tile_pool(name="xpool", bufs=2) as xpool,
            tc.tile_pool(name="hpool", bufs=1) as hpool,
            tc.tile_pool(name="tpool", bufs=4) as tpool,
            tc.tile_pool(name="ppool", bufs=2, space=bass.MemorySpace.PSUM) as ppool,
        ):
            # ---- persistent constants ----
            wih = [wpool.tile([128, 3 * H], dt.bfloat16, name=f"wih{c}", tag=f"wih{c}")
                   for c in range(4)]
            whh = [wpool.tile([128, 3 * H], dt.bfloat16, name=f"whh{c}", tag=f"whh{c}")
                   for c in range(4)]
            fcin = wpool.tile([F, H], dt.bfloat16, name="fcin", tag="fcin")
            # dedicated slice per block: staging DMAs carry no WAR/WAW deps
            stg = wpool.tile([F, T * BL], dt.bfloat16, name="stg", tag="stg")
            fcb = wpool.tile([128, 4], dt.float32, name="fcb", tag="fcb")
            brz = wpool.tile([128, 8], dt.float32, name="brz", tag="brz")
            bni = wpool.tile([128, 4], dt.float32, name="bni", tag="bni")
            bnh = wpool.tile([128, 4], dt.float32, name="bnh", tag="bnh")
            wout = wpool.tile([128, 32 * SPB], dt.bfloat16, name="wout", tag="wout")
            fcint = wpool.tile([F, H], dt.bfloat16, name="fcint", tag="fcint")
            h = [hpool.tile([128, BL], dt.float32, name=f"h{c}", tag=f"h{c}")
                 for c in range(4)]
            hb = [hpool.tile([128, BL], dt.bfloat16, name=f"hb{c}", tag=f"hb{c}")
                  for c in range(4)]

            CW = SPB * BL  # columns per block

            # early DMAs: block-0 inputs + projection weights first so PE can
            # start the block-0 projection while the big wih/whh DMAs land.
            nc.sync.dma_start(stg[:F, 0:CW], xdat_d[:, 0:CW])
            nc.sync.dma_start(fcint[:F, :], fcin_d[:])
            nc.sync.dma_start(fcb[:], fcb_d[:])
            # DVE funnel: PE Matmult supports only ONE embedded sem wait, so
            # route DMA-landed matmul operands through DVE; matmul deps then
            # collapse onto the single DVE semaphore.
            nc.vector.tensor_copy(fcin[:F, :], fcint[:F, :])
            nc.sync.dma_start(stg[:F, CW:2 * CW], xdat_d[:, CW:2 * CW])
            for c in range(4):
                nc.sync.dma_start(h[c][:], h0_d[c * 128:(c + 1) * 128, :])
                nc.scalar.activation(hb[c][:], h[c][:], AF.Copy)
            nc.sync.dma_start(brz[:], brz_d[:])
            nc.sync.dma_start(bni[:], bni_d[:])
            nc.sync.dma_start(bnh[:], bnh_d[:])
            nc.sync.dma_start(wout[:], wout_d[:])
            for c in range(4):
                nc.sync.dma_start(wih[c][:], wih_d[c * 128:(c + 1) * 128, :])
                nc.sync.dma_start(whh[c][:], whh_d[c * 128:(c + 1) * 128, :])

            def proj_col(off, xb_t, s):
                # reads the staging tile directly; _legalize splits the
                # resulting multi-wait matmuls onto PE NoOps
                for oc in range(4):
                    pj = ppool.tile([128, BL], dt.float32, name="gpj", tag="gx",
                                    bufs=3)
                    nc.tensor.matmul(pj[:], fcin[:F, oc * 128:(oc + 1) * 128],
                                     stg[:F, off + s * BL:off + (s + 1) * BL],
                                     start=True, stop=True)
                    nc.scalar.activation(xb_t[oc][:, s * BL:(s + 1) * BL], pj[:],
                                         AF.Relu, bias=fcb[:, oc:oc + 1])

            # block 0 projects its own inputs up front (PE filler during the
            # weight DMAs); later blocks are projected inside the prior block
            xb_cur = [xpool.tile([128, CW], dt.bfloat16, name=f"xb{oc}",
                                 tag=f"xb{oc}", bufs=2) for oc in range(4)]
            for s in range(SPB):
                proj_col(0, xb_cur, s)

            pend = None
            for blk in range(NBLK):
                if blk + 1 < NBLK:
                    xb_next = [xpool.tile([128, CW], dt.bfloat16, name=f"xb{oc}",
                                          tag=f"xb{oc}", bufs=2) for oc in range(4)]
                if blk + 2 < NBLK:
                    nc.sync.dma_start(stg[:F, (blk + 2) * CW:(blk + 3) * CW],
                                      xdat_d[:, (blk + 2) * CW:(blk + 3) * CW])
                yp = ppool.tile([SPB, BL], dt.float32, name="yp", tag="y", bufs=1)

                for s in range(SPB):
                    xsl = [xb_cur[c][:, s * BL:(s + 1) * BL] for c in range(4)]

                    # project next block's column first: pure filler, and the
                    # relus land in ACT's idle window ahead of the
                    # sigmoid -> hnb -> tanh chain
                    if blk + 1 < NBLK:
                        proj_col((blk + 1) * CW, xb_next, s)

                    # pass 0: n-gate input matmuls -> SBUF via DVE. Together
                    # with pass 1a these give PE ~3.4us of hb-independent
                    # filler covering the previous step's elementwise tail.
                    pis_list = []
                    for c4 in range(4):
                        mc = slice((8 + c4) * 128, (9 + c4) * 128)
                        pin = ppool.tile([128, BL], dt.float32, name="pin",
                                         tag="gx", bufs=3)
                        for c in range(4):
                            nc.tensor.matmul(pin[:], wih[c][:, mc], xsl[c],
                                             start=(c == 0), stop=(c == 3))
                        pis = tpool.tile([128, BL], dt.float32, name="pis",
                                         tag="pis", bufs=8)
                        nc.vector.tensor_copy(pis[:], pin[:])
                        pis_list.append(pis)

                    # r/z gates in two half-passes of 4 groups (PSUM budget:
                    # 4 grz + 3 gx + 1 y banks). Each half's input-side (wih)
                    # matmuls are emitted before the hb-dependent (whh) ones.
                    r_list, z_list = [], []
                    for half in range(2):
                        rzps = []
                        for m4 in range(4):
                            m = half * 4 + m4
                            mc = slice(m * 128, (m + 1) * 128)
                            ps = ppool.tile([128, BL], dt.float32, name="grz",
                                            tag="grz", bufs=4)
                            for c in range(4):
                                nc.tensor.matmul(ps[:], wih[c][:, mc], xsl[c],
                                                 start=(c == 0), stop=False)
                            rzps.append(ps)

                        # deferred output head for the previous step (hb still
                        # holds that step's state; not yet updated this step)
                        if half == 0 and pend is not None:
                            ypp, pb, psv = pend
                            for c in range(4):
                                w0 = (psv * 4 + c) * SPB
                                nc.tensor.matmul(
                                    ypp[:SPB, :], wout[:, w0:w0 + SPB], hb[c][:],
                                    start=(psv == 0 and c == 0),
                                    stop=(psv == SPB - 1 and c == 3))
                            if psv == SPB - 1:
                                ysb = tpool.tile([SPB, BL], dt.float32,
                                                 name="ysb", tag="ysb", bufs=2)
                                nc.scalar.activation(ysb[:SPB, :], ypp[:SPB, :],
                                                     AF.Copy)
                                nc.sync.dma_start(
                                    yout_d[pb * SPB:(pb + 1) * SPB, :],
                                    ysb[:SPB, :])
                            pend = None

                        for m4 in range(4):
                            m = half * 4 + m4
                            mc = slice(m * 128, (m + 1) * 128)
                            ps = rzps[m4]
                            for c in range(4):
                                nc.tensor.matmul(ps[:], whh[c][:, mc], hb[c][:],
                                                 start=False, stop=(c == 3))
                            g = tpool.tile([128, BL], dt.float32,
                                           name="rg" if m < 4 else "zg",
                                           tag="r" if m < 4 else "z", bufs=6)
                            nc.scalar.activation(g[:], ps[:], AF.Sigmoid,
                                                 bias=brz[:, m:m + 1])
                            (r_list if m < 4 else z_list).append(g)

                    # phase 1: ALL n-gate hidden matmuls read the OLD hb
                    # (updating hb inside this loop corrupts later chunks)
                    hnb_list = []
                    for c4 in range(4):
                        mc = slice((8 + c4) * 128, (9 + c4) * 128)
                        ph = ppool.tile([128, BL], dt.float32, name="gph",
                                        tag="gx", bufs=3)
                        for c in range(4):
                            nc.tensor.matmul(ph[:], whh[c][:, mc], hb[c][:],
                                             start=(c == 0), stop=(c == 3))
                        hnb = tpool.tile([128, BL], dt.float32, name="hnb",
                                         tag="hnb", bufs=8)
                        nc.scalar.activation(hnb[:], ph[:], AF.Identity,
                                             bias=bnh[:, c4:c4 + 1])
                        hnb_list.append(hnb)
                    # phase 2: elementwise updates (nothing here reads hb);
                    # h-update chain split DVE -> ACT -> Pool to shorten the
                    # critical path per engine
                    for c4 in range(4):
                        rhn = tpool.tile([128, BL], dt.float32, name="rhn",
                                         tag="rhn", bufs=3)
                        nc.vector.tensor_mul(rhn[:], r_list[c4][:],
                                             hnb_list[c4][:])
                        sa = tpool.tile([128, BL], dt.float32, name="sa",
                                        tag="sa", bufs=3)
                        nc.vector.tensor_add(sa[:], pis_list[c4][:], rhn[:])
                        nsb = tpool.tile([128, BL], dt.float32, name="nsb",
                                         tag="nsb", bufs=3)
                        nc.scalar.activation(nsb[:], sa[:], AF.Tanh,
                                             bias=bni[:, c4:c4 + 1])
                        dd = tpool.tile([128, BL], dt.float32, name="dd",
                                        tag="dd", bufs=3)
                        nc.gpsimd.tensor_sub(dd[:], h[c4][:], nsb[:])
                        zd = tpool.tile([128, BL], dt.float32, name="zd",
                                        tag="zd", bufs=3)
                        nc.gpsimd.tensor_mul(zd[:], z_list[c4][:], dd[:])
                        nc.gpsimd.tensor_add(h[c4][:], nsb[:], zd[:])
                        nc.scalar.activation(hb[c4][:], h[c4][:], AF.Copy)

                    pend = (yp, blk, s)

                if blk + 1 < NBLK:
                    xb_cur = xb_next

            # drain the final step's output head
            ypp, pb, psv = pend
            for c in range(4):
                w0 = (psv * 4 + c) * SPB
                nc.tensor.matmul(ypp[:SPB, :], wout[:, w0:w0 + SPB], hb[c][:],
                                 start=False, stop=(c == 3))
            ysb = tpool.tile([SPB, BL], dt.float32, name="ysb", tag="ysb", bufs=2)
            nc.scalar.activation(ysb[:SPB, :], ypp[:SPB, :], AF.Copy)
            nc.sync.dma_start(yout_d[pb * SPB:(pb + 1) * SPB, :], ysb[:SPB, :])

    _legalize(nc, mybir)
    return nc


def _get_nc():
    if "nc" not in _CACHE:
        _CACHE["nc"] = _build()
    return _CACHE["nc"]


def _wsp(w):
    chunks = w.reshape(4, 128)
    out = np.zeros((128, 32 * SPB), np.float32)
    for s in range(SPB):
        for c in range(4):
            out[:, (s * 4 + c) * SPB + s] = chunks[c]
    return out.astype(BF16)


def kernel(data, init, fc_in_W, fc_in_b, Wih, Whh, bih, bhh, fc_out_W, fc_out_b):
    from concourse.bass_utils import run_bass_kernel_spmd

    data = np.asarray(data, np.float32)
    init = np.asarray(init, np.float32)
    fc_in_W = np.asarray(fc_in_W, np.float32)
    fc_in_b = np.asarray(fc_in_b, np.float32)
    Wih = np.asarray(Wih, np.float32)
    Whh = np.asarray(Whh, np.float32)
    bih = np.asarray(bih, np.float32)
    bhh = np.asarray(bhh, np.float32)
    fc_out_W = np.asarray(fc_out_W, np.float32)
    fc_out_b = np.asarray(fc_out_b, np.float32)

    nc = _get_nc()

    in_maps = []
    for i in range(8):
        k, j = i // 2, i % 2
        d = data[j * BL:(j + 1) * BL]            # [256, 64, 64] (b,f,t)
        if k >= 2:
            d = d[:, :, ::-1]                    # reversed-time branches
        xdat = np.ascontiguousarray(d.transpose(1, 2, 0)).reshape(F, T * BL)
        brz = (bih[k][:2 * H] + bhh[k][:2 * H]).reshape(8, 128).T
        in_maps.append({
            "xdat": np.ascontiguousarray(xdat).astype(BF16),
            "fcin": np.ascontiguousarray(fc_in_W[k].T).astype(BF16),  # [64, 512]
            "fcb": np.ascontiguousarray(fc_in_b[k].reshape(4, 128).T),
            "wih": np.ascontiguousarray(Wih[k].T).astype(BF16),  # [512, 1536]
            "whh": np.ascontiguousarray(Whh[k].T).astype(BF16),
            "brz": np.ascontiguousarray(brz),
            "bni": np.ascontiguousarray(bih[k][2 * H:].reshape(4, 128).T),
            "bnh": np.ascontiguousarray(bhh[k][2 * H:].reshape(4, 128).T),
            "h0": np.ascontiguousarray(init[j * BL:(j + 1) * BL].T),
            "wout": _wsp(fc_out_W[k % 2]),
        })

    kw = {"trace": True} if TRACE else {}
    res = run_bass_kernel_spmd(nc, in_maps, list(range(8)), **kw)
    LAST["res"] = res
    y = [np.asarray(res.results[i]["yout"], np.float32) for i in range(8)]

    air_out = np.empty((B, T), np.float32)
    bed_out = np.empty((B, T), np.float32)
    for j in range(2):
        sl = slice(j * BL, (j + 1) * BL)
        air_out[sl] = (y[0 + j] + y[4 + j][::-1]).T + fc_out_b[0]
        bed_out[sl] = (y[2 + j] + y[6 + j][::-1]).T + fc_out_b[1]
    return air_out, bed_out

